# revision 57
# baseline (speedup 1.0000x reference)
"""Cross-attention kernel for Trainium2, sharded over 8 NeuronCores.

Sharding: rows of (B, S1) split 8 ways -> each core handles one batch's
half (2048 query rows) and recomputes that batch's small KV projection.
No collectives needed.

Host-side preprocessing (free - doesn't count toward HW time):
  - transpose x, y to feature-major, pad with a ones-row so the PE adds bq
  - transpose weights; per-head deinterleave permutation of the head_dim
    axis on the Q/K sides turns the reference's interleaved rotate_half
    into contiguous-half rotation
  - fold qn_w, kn_w into cos/sin tables; scale+rstd_q folded into the
    exp's per-partition scale
  - kn_b is dropped entirely: it shifts all scores of a row equally, which
    softmax cancels.

Device pipeline (v2): software-pipelined emission with a 3-deep stage skew
so the PE instruction stream never waits on the vector/act/dma chain:
iteration i emits Q-proj(i), scores(i-1) (interleaved with PV(i-2) and
out-proj(i-3) to let the Act exp drain the scores PSUM), per-head softmax
normalization folded into the PV-evacuation multiply, s-major ctx with a
single DMA transpose per chunk feeding the out-projection.
"""
import sys

sys.path.insert(0, '/opt/trn_rl_repo')

import numpy as np
import ml_dtypes

import concourse.bass as bass
import concourse.tile as tile
from concourse import bacc, mybir
from concourse.bass_utils import run_bass_kernel_spmd

bf16 = mybir.dt.bfloat16
f32 = mybir.dt.float32

# problem shapes (hardcoded per contest rules)
B, S1, S2, CQ, CKV, H, D = 4, 4096, 256, 1408, 1024, 16, 88
NCORES = 8
S = (B * S1) // NCORES          # 2048 query rows per core
NS = S // 128                   # 16 s-chunks
DP = 128                        # head_dim padded for transposes
G = 4                           # heads per RoPE/LN group (4*88 = 352)
NG = H // G
KC_Q = CQ // 128 + 1            # 12 contraction chunks (incl. bias ones-row)
KC_KV = CKV // 128              # 8
KC_O = CQ // 128                # 11
EPS = 1e-6
HALF = D // 2                   # 44

_BUILD_CACHE = {}


DBG = False


def _build(use_badd: bool, reps: int = 1):
    nc = bacc.Bacc("TRN2", target_bir_lowering=False)
    if DBG:
        dbg_ctx = nc.dram_tensor("dbg_ctx", [S, CQ], bf16, kind="ExternalOutput")
        dbg_qr = nc.dram_tensor("dbg_qr", [S, H * DP], bf16, kind="ExternalOutput")

    xT = nc.dram_tensor("xT", [128 * KC_Q, S], bf16, kind="ExternalInput")
    yT = nc.dram_tensor("yT", [CKV, S2], bf16, kind="ExternalInput")
    wq = nc.dram_tensor("wq", [128 * KC_Q, CQ], bf16, kind="ExternalInput")
    wkv = nc.dram_tensor("wkv", [CKV, 2 * CQ], bf16, kind="ExternalInput")
    wout = nc.dram_tensor("wout", [CQ, CQ], bf16, kind="ExternalInput")
    bkv = nc.dram_tensor("bkv", [2 * CQ], bf16, kind="ExternalInput")
    bout = nc.dram_tensor("bout", [CQ], bf16, kind="ExternalInput")
    csw = nc.dram_tensor("csw", [S, 2 * DP], bf16, kind="ExternalInput")
    if use_badd:
        badd = nc.dram_tensor("badd", [S, DP], bf16, kind="ExternalInput")
    out = nc.dram_tensor("out", [S, CQ], f32, kind="ExternalOutput")

    # kv projection output tiling: 4 k-groups of 352, then v in 4x352
    k_tiles = [(g * 352, 352) for g in range(NG)]
    v_tiles = [(CQ + g * 352, 352) for g in range(NG)]
    o_tiles = [(0, 512), (512, 512), (1024, 384)]
    q_tiles = o_tiles

    with tile.TileContext(nc) as tc:
        with (
            tc.tile_pool(name="persist", bufs=1) as persist,
            tc.tile_pool(name="xq", bufs=3) as xqp,
            tc.tile_pool(name="cs", bufs=3) as csp,
            tc.tile_pool(name="kwork", bufs=2) as kworkp,
            tc.tile_pool(name="qwork", bufs=2) as qwork,
            tc.tile_pool(name="stats", bufs=2) as statsp,
            tc.tile_pool(name="outsb", bufs=2) as outsbp,
            tc.tile_pool(name="ps_q", bufs=3, space="PSUM") as ps_q,
            tc.tile_pool(name="ps_mix", bufs=5, space="PSUM") as ps_mix,
        ):
            # ---------- persistent tiles / weight loads ----------
            # yT first: the first KV matmul gates kernel start
            yT_sb = persist.tile([128, KC_KV, S2], bf16, tag="yT_sb")
            nc.scalar.dma_start(yT_sb[:], yT[:].rearrange("(k p) t -> p k t", p=128))
            wq_sb = persist.tile([128, KC_Q, CQ], bf16, tag="wq_sb")

            def emit_wq_loads():
                for _g in range(NG):
                    nc.sync.dma_start(
                        wq_sb[:, :, _g * 352:(_g + 1) * 352],
                        wq[:].rearrange("(k p) o -> p k o", p=128)
                        [:, :, _g * 352:(_g + 1) * 352])

            bkv_ap = bkv[:]
            bkv_bc = persist.tile([128, 2 * CQ], bf16, tag="bkv_bc")
            nc.gpsimd.dma_start(bkv_bc[:], bass.AP(
                tensor=bkv_ap.tensor, offset=bkv_ap.offset,
                ap=[[0, 128], *bkv_ap.ap]))
            bout_ap = bout[:]
            bout_bc = persist.tile([128, CQ], bf16, tag="bout_bc")
            nc.gpsimd.dma_start(bout_bc[:], bass.AP(
                tensor=bout_ap.tensor, offset=bout_ap.offset,
                ap=[[0, 128], *bout_ap.ap]))

            wout_sb = persist.tile([128, KC_O, CQ], bf16, tag="wout_sb")

            def emit_rsqrt(y, v_ap, n, post_scale=None, eng=None):
                # y = 1/sqrt(v + EPS) via Newton iterations (tiny ops)
                eng = eng or nc.vector
                eng.tensor_scalar(
                    out=y, in0=v_ap, scalar1=-0.5, scalar2=1.5 + EPS,
                    op0=mybir.AluOpType.mult, op1=mybir.AluOpType.add)
                eng.tensor_scalar_max(out=y, in0=y, scalar1=0.08)
                t1 = statsp.tile([128, n], f32, tag="nr_t1")
                for _ in range(3):
                    eng.tensor_mul(t1[:], y, y)
                    eng.tensor_tensor(t1[:], t1[:], v_ap,
                                      mybir.AluOpType.mult)
                    eng.tensor_scalar(
                        out=t1[:], in0=t1[:], scalar1=-0.5, scalar2=1.5 + 0.5 * EPS,
                        op0=mybir.AluOpType.mult, op1=mybir.AluOpType.add)
                    eng.tensor_mul(y, y, t1[:])
                if post_scale is not None:
                    eng.tensor_scalar_mul(out=y, in0=y,
                                          scalar1=post_scale)

            kln = [persist.tile([128, H, DP], bf16, tag=f"kln{t}", name=f"kln{t}")
                   for t in range(2)]
            # kT layout: [d_pad, head, t]
            kT = persist.tile([128, H, S2], bf16, tag="kT")
            # v in per-head slots of 96 with a ones-column at [.., 88]:
            # the PV matmul then produces the softmax denominator for free
            v_sb = persist.tile([128, 2, H, 96], bf16, tag="v_sb")

            # manual ring buffers for the deep-skewed pipelined main loop
            qrope = [persist.tile([128, H, DP], bf16, tag=f"qrope{j}",
                                  name=f"qrope{j}") for j in range(2)]
            qropeT = [persist.tile([128, H, 128], bf16, tag=f"qropeT{j}",
                                   name=f"qropeT{j}") for j in range(3)]
            attn = [persist.tile([128, H, S2], bf16, tag=f"attn{j}",
                                 name=f"attn{j}") for j in range(2)]
            aT = [persist.tile([128, 2 * H, 128], bf16, tag=f"aT{j}",
                               name=f"aT{j}") for j in range(3)]
            ctx_sm = [persist.tile([128, CQ], bf16, tag=f"ctx{j}",
                                   name=f"ctx{j}") for j in range(2)]
            ctxT = [persist.tile([128, KC_O, 128], bf16, tag=f"ctxT{j}",
                                 name=f"ctxT{j}") for j in range(3)]
            # 1/128 constant block: the bias rank-1 update runs as a
            # K=128 matmul so the PE tile geometry matches the group
            ones128 = persist.tile([128, 128], bf16, tag="ones128")

            def emit_body(rep):
                # zero the rope pad columns once (they feed the transposed
                # contraction rows, which must be 0)
                if rep == 0:
                    for t in range(2):
                        nc.gpsimd.memset(kln[t][:, :, D:DP], 0.0)
                        nc.gpsimd.memset(qrope[t][:, :, D:DP], 0.0)
                    nc.gpsimd.memset(ones128[:], 1.0 / 128.0)
                    nc.gpsimd.memset(v_sb[:, :, :, D:D + 1], 1.0)

                # prefetch the first x / cos-sin chunks before the KV
                # weight traffic so Q(0) isn't gated on them
                def stage_q_load(p):
                    # loads x for chunk pair (2p, 2p+1): 512B runs, full rate
                    xq = xqp.tile([128, KC_Q, 256], bf16, tag="xq")
                    nc.sync.dma_start(
                        xq[:], xT[:].rearrange("(k p) s -> p k s", p=128)
                        [:, :, p * 256:(p + 1) * 256])
                    return xq

                def stage_cs_load(i):
                    csw_sb = csp.tile([128, 2 * DP], bf16, tag="csw")
                    nc.sync.dma_start(csw_sb[:], csw[i * 128:(i + 1) * 128, :])
                    ba_sb = None
                    if use_badd:
                        ba_sb = csp.tile([128, DP], bf16, tag="ba")
                        nc.sync.dma_start(ba_sb[:], badd[i * 128:(i + 1) * 128, :])
                    return (csw_sb, ba_sb)

                xloads = {0: stage_q_load(0), 1: stage_q_load(1)}
                csloads = {0: stage_cs_load(0), 1: stage_cs_load(1)}

                qstate = {}

                def stage_q_mm(i):
                    xq = xloads[i // 2]
                    s0 = (i % 2) * 128
                    # three 512-col psum tiles from a 3-deep pool: group g of
                    # chunk i+1 only WARs group g of chunk i (evacuated early)
                    psq = [ps_q.tile([128, 512], f32, tag="psq", name=f"psq{_t}")
                           for _t in range(3)]
                    for ti, (o0, ow) in enumerate(q_tiles):
                        for kc in range(KC_Q):
                            nc.tensor.matmul(
                                psq[ti][:, :ow],
                                xq[:, kc, s0:s0 + 128],
                                wq_sb[:, kc, o0:o0 + ow],
                                start=(kc == 0), stop=(kc == KC_Q - 1))
                    qstate[i] = psq

                # ---------- KV phase ----------
                for ti_kv, (o0, ow) in enumerate(k_tiles + v_tiles):
                    # stage the kv weights in the (idle during KV) attn
                    # ring tiles to save SBUF
                    wkv_t = attn[ti_kv % 2][:].rearrange(
                        "p h t -> p (h t)")[:, 0:KC_KV * 352].rearrange(
                        "p (k o) -> p k o", o=352)
                    nc.sync.dma_start(
                        wkv_t[:, :, :ow],
                        wkv[:].rearrange("(k p) o -> p k o", p=128)[:, :, o0:o0 + ow])
                    if ti_kv == 0 and rep == 0:
                        emit_wq_loads()
                    if ti_kv == 2:
                        # give the PE a full Q chunk to chew while the KV
                        # LN chain drains on DVE
                        stage_q_mm(0)
                    for t in range(2):
                        ps = ps_mix.tile([128, 512], f32, tag="mix", name="kvps")
                        for kc in range(KC_KV):
                            nc.tensor.matmul(
                                ps[:, :ow],
                                yT_sb[:, kc, t * 128:(t + 1) * 128],
                                wkv_t[:, kc, :ow],
                                start=(kc == 0), stop=(kc == KC_KV - 1))
                        if o0 < CQ:
                            g0 = o0 // 352 * G
                            kb = kworkp.tile([128, G, D], f32, tag="kb")
                            nc.vector.tensor_tensor(
                                kb[:].rearrange("p g d -> p (g d)"), ps[:, :ow],
                                bkv_bc[:, o0:o0 + ow], mybir.AluOpType.add)
                            st = statsp.tile([128, G, 6], f32, tag="st")
                            mv = statsp.tile([128, G, 2], f32, tag="mv")
                            for g in range(G):
                                nc.vector.bn_stats(st[:, g, :], kb[:, g, :])
                                nc.vector.bn_aggr(mv[:, g, :], st[:, g, :])
                            rstd = statsp.tile([128, G], f32, tag="rstd")
                            emit_rsqrt(rstd[:], mv[:, :, 1], G)
                            for g in range(G):
                                nc.gpsimd.tensor_scalar(
                                    out=kln[t][:, g0 + g, 0:D], in0=kb[:, g, :],
                                    scalar1=mv[:, g, 0:1], scalar2=rstd[:, g:g + 1],
                                    op0=mybir.AluOpType.subtract,
                                    op1=mybir.AluOpType.mult)
                        else:
                            vg = (o0 - CQ) // 352
                            nc.vector.tensor_tensor(
                                v_sb[:, t, vg * G:(vg + 1) * G, 0:D],
                                ps[:, :ow].rearrange("p (g d) -> p g d", d=D),
                                bkv_bc[:, o0:o0 + ow]
                                .rearrange("p (g d) -> p g d", d=D),
                                mybir.AluOpType.add)
                for t in range(2):
                    nc.sync.dma_start_transpose(
                        kT[:, :, t * 128:(t + 1) * 128],
                        kln[t][:].rearrange("p h d -> p (h d)"))

                if rep == 0:
                    # out-proj weights aren't needed until iteration 3;
                    # issue the load behind the KV phase traffic
                    nc.scalar.dma_start(
                        wout_sb[:], wout[:].rearrange("(k p) o -> p k o", p=128))

                # ---------- pipelined main loop ----------
                # iteration i: Qmm(i) | zip{scores(i-1), PV(i-2), out(i-3)}
                # | Qpost(i) -- the post-processing (stats/rope) is emitted
                # AFTER the zip so each engine's queue order matches data
                # readiness
                qstate = {}
                rstds = {}


                def stage_q_mm(i):
                    xq = xloads[i // 2]
                    s0 = (i % 2) * 128
                    # three 512-col psum tiles from a 3-deep pool: group g of
                    # chunk i+1 only WARs group g of chunk i (evacuated early)
                    psq = [ps_q.tile([128, 512], f32, tag="psq", name=f"psq{_t}")
                           for _t in range(3)]
                    for ti, (o0, ow) in enumerate(q_tiles):
                        for kc in range(KC_Q):
                            nc.tensor.matmul(
                                psq[ti][:, :ow],
                                xq[:, kc, s0:s0 + 128],
                                wq_sb[:, kc, o0:o0 + ow],
                                start=(kc == 0), stop=(kc == KC_Q - 1))
                    qstate[i] = psq

                def stage_q_post(i):
                    psq = qstate.pop(i)
                    csw_sb, ba_sb = csloads.pop(i)
                    st = statsp.tile([128, H, 2, 6], f32, tag="qst")
                    mv_all = statsp.tile([128, H, 2], f32, tag="mv_all")
                    rstd_all = statsp.tile([128, H], f32, tag="rstd_all")

                    def emit_stats(h):
                        c0 = h * D
                        ti, o = divmod(c0, 512)
                        if o + D <= 512:
                            nc.vector.bn_stats(st[:, h, 0, :], psq[ti][:, o:o + D])
                        else:
                            # straddling head: bn_aggr's variance combine
                            # assumes equal-count groups, so gather the two
                            # psum pieces into one contiguous scratch first
                            n1 = 512 - o
                            hsc = qwork.tile([128, D], f32, tag="hsc")
                            nc.vector.tensor_copy(hsc[:, 0:n1], psq[ti][:, o:512])
                            nc.vector.tensor_copy(hsc[:, n1:D],
                                                  psq[ti + 1][:, 0:D - n1])
                            nc.vector.bn_stats(st[:, h, 0, :], hsc[:])
                        nc.vector.bn_aggr(mv_all[:, h, :], st[:, h, 0, :])

                    def emit_rope_group(g):
                        # (q - mean) * rstd (fused, so exp runs at scale=1)
                        # + rope for heads 4g..4g+3
                        gs = slice(g * G, (g + 1) * G)
                        qcr = qwork.tile([128, G, D], bf16, tag="qcr")
                        for hh in range(G):
                            h = g * G + hh
                            ti, o = divmod(h * D, 512)
                            if o + D <= 512:
                                nc.vector.tensor_scalar(
                                    out=qcr[:, hh, :], in0=psq[ti][:, o:o + D],
                                    scalar1=mv_all[:, h, 0:1],
                                    scalar2=rstd_all[:, h:h + 1],
                                    op0=mybir.AluOpType.subtract,
                                    op1=mybir.AluOpType.mult)
                            else:
                                n1 = 512 - o
                                nc.vector.tensor_scalar(
                                    out=qcr[:, hh, 0:n1], in0=psq[ti][:, o:512],
                                    scalar1=mv_all[:, h, 0:1],
                                    scalar2=rstd_all[:, h:h + 1],
                                    op0=mybir.AluOpType.subtract,
                                    op1=mybir.AluOpType.mult)
                                nc.vector.tensor_scalar(
                                    out=qcr[:, hh, n1:D],
                                    in0=psq[ti + 1][:, 0:D - n1],
                                    scalar1=mv_all[:, h, 0:1],
                                    scalar2=rstd_all[:, h:h + 1],
                                    op0=mybir.AluOpType.subtract,
                                    op1=mybir.AluOpType.mult)
                        tt = qwork.tile([128, G, D], bf16, tag="tt")
                        nc.gpsimd.tensor_mul(
                            tt[:, :, 0:HALF], qcr[:, :, HALF:D],
                            csw_sb[:, None, DP:DP + HALF]
                            .to_broadcast([128, G, HALF]))
                        nc.gpsimd.tensor_mul(
                            tt[:, :, HALF:D], qcr[:, :, 0:HALF],
                            csw_sb[:, None, DP + HALF:DP + D]
                            .to_broadcast([128, G, HALF]))
                        u = qwork.tile([128, G, D], bf16, tag="u")
                        nc.vector.tensor_mul(
                            u[:], qcr[:],
                            csw_sb[:, None, 0:D].to_broadcast([128, G, D]))
                        if use_badd:
                            nc.vector.tensor_add(u[:], u[:], ba_sb[:, None, 0:D]
                                                 .to_broadcast([128, G, D]))
                        nc.vector.tensor_add(qrope[i % 2][:, gs, 0:D],
                                             u[:], tt[:])
                        if g % 2 == 1:
                            h0 = (g - 1) * G
                            nc.sync.dma_start_transpose(
                                qropeT[i % 3][:, h0:h0 + 2 * G, :],
                                qrope[i % 2][:, h0:h0 + 2 * G, :]
                                .rearrange("p h d -> p (h d)"))

                    for h in range(0, 8):
                        emit_stats(h)
                    # rstd on the Pool engine (folds the 1/sqrt(D) scale)
                    emit_rsqrt(rstd_all[:, 0:8], mv_all[:, 0:8, 1], 8,
                               post_scale=float(D) ** -0.5, eng=nc.gpsimd)
                    for h in range(8, 16):
                        emit_stats(h)
                    emit_rsqrt(rstd_all[:, 8:16], mv_all[:, 8:16, 1], 8,
                               post_scale=float(D) ** -0.5, eng=nc.gpsimd)
                    emit_rope_group(0)
                    emit_rope_group(1)
                    emit_rope_group(2)
                    emit_rope_group(3)
                    if DBG:
                        nc.sync.dma_start(
                            dbg_qr[i * 128:(i + 1) * 128, :],
                            qrope[i % 2][:].rearrange("p h d -> p (h d)"))

                def emit_score_pair(j, m):
                    # scores + exp for head pair (2m, 2m+1); rstd is folded
                    # into q so the exp runs one op per pair at scale=1
                    ssc = ps_mix.tile([128, 2, S2], f32, tag="mix", name="ssc")
                    for hh in range(2):
                        h = 2 * m + hh
                        nc.tensor.matmul(ssc[:, hh, :], qropeT[j % 3][:, h, :],
                                         kT[:, h, :], start=True, stop=True)
                    nc.scalar.activation(
                        out=attn[j % 2][:, 2 * m:2 * m + 2, :], in_=ssc[:],
                        func=mybir.ActivationFunctionType.Exp)
                    if m % 2 == 1:
                        # transpose 4 heads as soon as their exps land, so
                        # the first PV group next iteration is never gated
                        h0 = 2 * (m - 1)
                        nc.sync.dma_start_transpose(
                            aT[j % 3][:, 2 * h0:2 * h0 + 8, :],
                            attn[j % 2][:, h0:h0 + 4, :]
                            .rearrange("p h t -> p (h t)"))

                def emit_pv_group(k, g):
                    pvt = ps_mix.tile([128, G, 128], f32, tag="mix", name="pvt")
                    for hh in range(G):
                        h = g * G + hh
                        for t in range(2):
                            # rhs col 88 is the ones-column: the denominator
                            # accumulates into pvt[:, hh, 88]
                            nc.tensor.matmul(
                                pvt[:, hh, 0:D + 1],
                                aT[k % 3][:, 2 * h + t, :],
                                v_sb[:, t, h, 0:D + 1],
                                start=(t == 0), stop=(t == 1))
                    # normalize by the accumulated denominator while
                    # evacuating to SBUF (DVE: gpsimd cannot touch PSUM)
                    rd = statsp.tile([128, G], f32, tag="rd", bufs=4)
                    nc.vector.reciprocal(rd[:], pvt[:, :, D])
                    nc.vector.tensor_tensor(
                        ctx_sm[k % 2][:, g * 352:(g + 1) * 352]
                        .rearrange("p (g d) -> p g d", d=D),
                        pvt[:, :, 0:D],
                        rd[:, :, None].to_broadcast([128, G, D]),
                        mybir.AluOpType.mult)
                    if g == NG - 1:
                        nc.sync.dma_start_transpose(ctxT[k % 3][:],
                                                     ctx_sm[k % 2][:])
                        if DBG:
                            nc.sync.dma_start(
                                dbg_ctx[k * 128:(k + 1) * 128, :],
                                ctx_sm[k % 2][:])

                def emit_out_tile(ll, ti):
                    (o0, ow) = o_tiles[ti]
                    pso = ps_mix.tile([128, 512], f32, tag="mix", name="pso")
                    for c in range(KC_O):
                        nc.tensor.matmul(
                            pso[:, :ow],
                            ctxT[ll % 3][:, c, :],
                            wout_sb[:, c, o0:o0 + ow],
                            start=(c == 0), stop=False)
                    # + bout via a (1/128)*ones matmul against the
                    # broadcast bias tile (no bias-add op on DVE)
                    nc.tensor.matmul(
                        pso[:, :ow], ones128[:],
                        bout_bc[:, o0:o0 + ow],
                        start=False, stop=True)
                    osb = outsbp.tile([128, 512], f32, tag="outsb")
                    if ti == 1:
                        nc.vector.tensor_copy(osb[:, :ow], pso[:, :ow])
                    else:
                        nc.scalar.copy(osb[:, :ow], pso[:, :ow])
                    nc.sync.dma_start(
                        out[ll * 128:(ll + 1) * 128, o0:o0 + ow], osb[:, :ow])

                for i in range(NS + 6):
                    if 0 < i < NS:
                        stage_q_mm(i)
                    j, k, ll = i - 2, i - 4, i - 6
                    do_j = 0 <= j < NS
                    do_k = 0 <= k < NS
                    do_l = 0 <= ll < NS

                    def sc(m):
                        if do_j:
                            emit_score_pair(j, m)

                    # interleave score pairs with PV groups and out tiles;
                    # the shared 5-deep psum ring gives every allocation
                    # multiple slots of WAR slack
                    sc(0)
                    if do_k:
                        emit_pv_group(k, 0)
                    sc(1)
                    if do_k:
                        emit_pv_group(k, 1)
                    sc(2)
                    if do_l:
                        emit_out_tile(ll, 0)
                    sc(3)
                    if do_k:
                        emit_pv_group(k, 2)
                    sc(4)
                    if do_k:
                        emit_pv_group(k, 3)
                    sc(5)
                    if do_l:
                        emit_out_tile(ll, 1)
                    sc(6)
                    if do_l:
                        emit_out_tile(ll, 2)
                    sc(7)
                    if i < NS:
                        stage_q_post(i)
                    if i % 2 == 0 and (i + 4) // 2 < (NS + 1) // 2:
                        xloads[(i + 4) // 2] = stage_q_load((i + 4) // 2)
                    if i + 2 < NS:
                        csloads[i + 2] = stage_cs_load(i + 2)

            for _rep in range(reps):
                emit_body(_rep)

    nc.finalize()
    return nc


def _prep(inputs):
    """Host-side shared (per-core independent parts built in kernel())."""
    x = np.asarray(inputs['x'], np.float32)
    y = np.asarray(inputs['y'], np.float32)
    cos = np.asarray(inputs['cos'], np.float32)
    sin = np.asarray(inputs['sin'], np.float32)
    Wq = np.asarray(inputs['Wq'], np.float32)
    bq = np.asarray(inputs['bq'], np.float32)
    Wkv = np.asarray(inputs['Wkv'], np.float32)
    bkv = np.asarray(inputs['bkv'], np.float32)
    qn_w = np.asarray(inputs['qn_w'], np.float32)
    qn_b = np.asarray(inputs['qn_b'], np.float32)
    kn_w = np.asarray(inputs['kn_w'], np.float32)
    kn_b = np.asarray(inputs['kn_b'], np.float32)  # noqa: F841  (cancels in softmax)
    Wout = np.asarray(inputs['Wout'], np.float32)
    bout = np.asarray(inputs['bout'], np.float32)

    perm = np.concatenate([np.arange(0, D, 2), np.arange(1, D, 2)])
    swapv = np.concatenate([np.arange(HALF, D), np.arange(0, HALF)])
    sign = np.concatenate([-np.ones(HALF, np.float32), np.ones(HALF, np.float32)])

    # Q weights: permute head_dim within each head, transpose, append bias row
    Wq_p = Wq.reshape(H, D, CQ)[:, perm, :].reshape(CQ, CQ)
    bq_p = bq.reshape(H, D)[:, perm].reshape(CQ)
    wq_ext = np.zeros((128 * KC_Q, CQ), np.float32)
    wq_ext[:CQ] = Wq_p.T
    wq_ext[CQ] = bq_p

    # KV: permute k-half head_dim (bias too), transpose
    Wkv_p = Wkv.reshape(2, H, D, CKV).copy()
    Wkv_p[0] = Wkv_p[0][:, perm, :]
    bkv_p = bkv.reshape(2, H, D).copy()
    bkv_p[0] = bkv_p[0][:, perm]
    wkvT = Wkv_p.reshape(2 * CQ, CKV).T.copy()
    bkv_p = bkv_p.reshape(2 * CQ)

    wq_vec = qn_w[perm]
    wk_vec = kn_w[perm]
    bq_ln = qn_b[perm]

    cos_p = cos[:, perm]
    sin_p = sin[:, perm]
    wfold = wq_vec * wk_vec
    CW = cos_p * wfold[None, :]                                   # [S1, D]
    SW = sign[None, :] * sin_p * (wq_vec[swapv] * wk_vec)[None, :]
    use_badd = bool(np.any(bq_ln != 0.0))
    BA = wk_vec[None, :] * (bq_ln[None, :] * cos_p
                            + sign[None, :] * bq_ln[swapv][None, :] * sin_p)

    return dict(
        x=x, y=y, wq_ext=wq_ext, wkvT=wkvT, bkv_p=bkv_p,
        woutT=Wout.T.copy(), bout=bout, CW=CW, SW=SW, BA=BA,
        use_badd=use_badd)


def _make_in_maps(p):
    use_badd = p['use_badd']
    wq_bf = p['wq_ext'].astype(ml_dtypes.bfloat16)
    wkv_bf = p['wkvT'].astype(ml_dtypes.bfloat16)
    wout_bf = p['woutT'].astype(ml_dtypes.bfloat16)
    in_maps = []
    for c in range(NCORES):
        b = c // 2
        s0 = (c % 2) * S
        xTe = np.zeros((128 * KC_Q, S), np.float32)
        xTe[:CQ] = p['x'][b, s0:s0 + S].T
        xTe[CQ] = 1.0
        cswp = np.zeros((S, 2 * DP), np.float32)
        cswp[:, :D] = p['CW'][s0:s0 + S]
        cswp[:, DP:DP + D] = p['SW'][s0:s0 + S]
        m = {
            'xT': xTe.astype(ml_dtypes.bfloat16),
            'yT': p['y'][b].T.astype(ml_dtypes.bfloat16).copy(),
            'wq': wq_bf, 'wkv': wkv_bf, 'wout': wout_bf,
            'bkv': p['bkv_p'].astype(ml_dtypes.bfloat16),
            'bout': p['bout'].astype(ml_dtypes.bfloat16),
            'csw': cswp.astype(ml_dtypes.bfloat16),
        }
        if use_badd:
            bap = np.zeros((S, DP), np.float32)
            bap[:, :D] = p['BA'][s0:s0 + S]
            m['badd'] = bap.astype(ml_dtypes.bfloat16)
        in_maps.append(m)
    return in_maps


def get_nc(use_badd, reps=1):
    key = (use_badd, reps)
    if key not in _BUILD_CACHE:
        _BUILD_CACHE[key] = _build(use_badd, reps)
    return _BUILD_CACHE[key]


def kernel(**inputs) -> np.ndarray:
    p = _prep(inputs)
    in_maps = _make_in_maps(p)
    nc = get_nc(p['use_badd'])
    res = run_bass_kernel_spmd(nc, in_maps, core_ids=list(range(NCORES)))
    outp = np.empty((B, S1, CQ), np.float32)
    for c in range(NCORES):
        b = c // 2
        s0 = (c % 2) * S
        outp[b, s0:s0 + S] = res.results[c]['out']
    return outp
